# revision 20
# baseline (speedup 1.0000x reference)
"""MultiHeadGAT Trainium2 kernel — 8 NeuronCores, raw Bass.

Sharding: 8 cores = 4 head-pairs x 2 node-halves.
Core c handles heads {2g, 2g+1} (g = c//2) and destination nodes
i in half ih = c%2. Per-core node order is permuted so the core's
own i-half is always nodes [0, 1536) (compile-time-uniform SPMD).

Math (per head), exploiting softmax column-scale invariance:
  v_ij = f_src[i] + f_dst[j];  e = exp(lrelu(v, 0.2));  att = masked softmax_j
  exp(lrelu(v)) = max(exp(v), exp(0.2 v)).  Scaling the attention matrix by
  any per-i factor cancels in softmax; with c[i] = exp(f_src[i]):
    P[j,i] ~ Ed[j] * adj[j,i] * max(1, Gs[i]*G[j])
  where Ed = exp(f_dst), G = exp(-0.8 f_dst), Gs = exp(-0.8 f_src).
  Ed[j] folds into the aggregation weights h_s[j,:] = h[j,:]*Ed[j], with an
  extra Ed column producing the softmax denominator during the same PE
  accumulation:
    [num[o,i]; D[i]] = sum_j [h_s; Ed][j,:]^T * Q[j,i],  Q = adj*max(1,Gs*G)
    out_T[o,i] = num[o,i] / D[i]

Engine plan per core (raw bass, manual semaphores):
  PE : proj h_T = W^T x_T (per head), f_src/f_dst column matmuls,
       aggregation (bf16), 1/D row broadcast via ones outer-product
  DVE: psum->sbuf copies, hs scaling, u = max(Gs_bcast*G[j], 1) (4x ts),
       Q = u*adj (2x tt) for its share of chunks, reciprocal, final divide
  GPS: Q = u*adj for its share of chunks (parallel with DVE)
  ACT: psum->sbuf copies, exps, hs transposes (HWDGE ring), gs bounce dmas,
       D-row copies, rbc psum->sbuf copies
  SP : bulk loads (xt, adj), output stores
"""

import numpy as np
import ml_dtypes

import concourse.bass as bass
import concourse.mybir as mybir
from concourse.bass_utils import run_bass_kernel_spmd

BF16 = mybir.dt.bfloat16
F32 = mybir.dt.float32

N = 3072          # nodes
KD = 512          # in dim
OD = 64           # out dim
OD1 = OD + 1
HSS = 80          # hs chunk stride (32B-aligned for dma transpose)
HEADS = 8
H2 = 2            # heads per core
IH = 1536         # i-half per core
JC = 24           # j chunks of 128
KC = 4            # k chunks of 128
IBH = 3           # i blocks of 512 in a half
NBA = 24          # adj slots (fully resident)
NBU = 8           # u ring slots
NBQ = 8           # q ring slots
H1P = 12          # emission index for h1 exps/gsb inserts
N_CORES = 8

# Q-mul engine split: chunk (by emission index) runs on GPS iff e % MOD < REM
GPS_MOD, GPS_REM = 3, 1
# tail insertion points (emission indices) for head-0 normalize pipeline
E_DC0 = 25      # ACT D-row copies
E_RC0 = 26      # DVE reciprocal
E_BC0 = 28      # PE bcast MMs (b at E_BC0+b)
E_RB0 = 29      # ACT rbc copies (b at E_RB0+b)
E_MU0 = 36      # DVE final mul
E_HT1 = 3       # DVE hT1 copy insertion
H1CP = 1        # ACT hT1 copy insertion


def _chunk_order():
    """Emission order: all of h0, then all of h1."""
    return [(h, jc) for h in range(H2) for jc in range(JC)]


def build_program(debug=False):
    nc = bass.Bass()

    xt = nc.declare_dram_parameter("xt", [KD, N], BF16, isOutput=False)
    adjt = nc.declare_dram_parameter("adjt", [N, IH], BF16, isOutput=False)
    wp = nc.declare_dram_parameter("wp", [H2, KD, OD], BF16, isOutput=False)
    av = nc.declare_dram_parameter("av", [OD, H2], BF16, isOutput=False)
    asb = nc.declare_dram_parameter("asb", [OD, H2 * 128], BF16, isOutput=False)
    out = nc.declare_dram_parameter("out", [H2, OD, IH], F32, isOutput=True)

    dbg = {}
    if debug:
        dbg["hT"] = nc.declare_dram_parameter("dbg_hT", [OD, H2 * N], BF16, isOutput=True)
        dbg["g"] = nc.declare_dram_parameter("dbg_g", [128, H2 * JC], F32, isOutput=True)
        dbg["ed"] = nc.declare_dram_parameter("dbg_ed", [128, H2 * JC], F32, isOutput=True)
        dbg["gsb"] = nc.declare_dram_parameter("dbg_gsb", [128, H2 * IH], BF16, isOutput=True)
        dbg["hs"] = nc.declare_dram_parameter("dbg_hs", [128, H2 * JC * HSS], BF16, isOutput=True)
        dbg["q"] = nc.declare_dram_parameter("dbg_q", [128, NBQ * IH], BF16, isOutput=True)
        dbg["rec"] = nc.declare_dram_parameter("dbg_rec", [OD, H2 * IH], F32, isOutput=True)

    CH = _chunk_order()
    e_of = {hc: e for e, hc in enumerate(CH)}
    NE = len(CH)

    def is_gps(e):
        return (e % GPS_MOD) < GPS_REM

    # ---------------- static schedule pre-pass: semaphore counts ----------
    # sem_sp: SP ring dmas (16 each): wp, av, asb, xt*8, adj*24, out*2
    # xt split: (kc, half) pairs ordered half-outer
    sp_xt_done = {}
    nsp = 3
    for half in range(2):
        nsp += 1
        for kc in range(KC):
            sp_xt_done[(kc, half)] = 16 * nsp
    ADJB = 6   # adj chunks per batched dma
    sp_adj_done = [16 * (5 + jc // ADJB + 1) for jc in range(JC)]

    # sem_tp: ACT ring dmas (16 each): one whole-head transpose per head
    tp_trans = [16, 32]

    # sem_pe (inc 1): proj0, fcols0, fsbc0, proj1, fcols1, fsbc1,
    # agg per emission with h0-bcast MMs inserted at E_BC0+b, h1 bcasts last
    pe_proj = [1, 4]
    pe_fcols = [2, 5]
    pe_fsbc = [3, 6]
    pe_agg = {}
    pe_bcast = {}
    cnt = 6
    for e in range(NE):
        cnt += 1
        pe_agg[e] = cnt
        for b in range(IBH):
            if e == E_BC0 + b:
                cnt += 1
                pe_bcast[(0, b)] = cnt
    for b in range(IBH):
        cnt += 1
        pe_bcast[(1, b)] = cnt

    # sem_act (inc 1), ACT stream order: hT0, exps0(g/ed), gsb0, then per-chunk
    # hs incs with inserts: [hT1, exps1, gsb1] at H1P, dcopy0 at E_DC0,
    # rbc0 blocks at E_RB0+b; after loop: dcopy1, rbc1 blocks
    act_hT = [1, 0]
    act_exps = [2, 0]
    act_gsb = [3, 0]
    act_hs = {}
    act_dcopy = [0, 0]
    act_rbc = {}
    cnt_a = 3
    for e in range(NE):
        if e == H1CP:
            cnt_a += 1; act_hT[1] = cnt_a
        if e == H1P:
            cnt_a += 1; act_exps[1] = cnt_a
            cnt_a += 1; act_gsb[1] = cnt_a
        if e == E_DC0:
            cnt_a += 1; act_dcopy[0] = cnt_a
        for b in range(IBH):
            if e == E_RB0 + b:
                cnt_a += 1; act_rbc[(0, b)] = cnt_a
        cnt_a += 1
        act_hs[e] = cnt_a
    cnt_a += 1; act_dcopy[1] = cnt_a
    for b in range(IBH):
        cnt_a += 1
        act_rbc[(1, b)] = cnt_a

    # sem_dve (inc 1): hT0, then per emission u-inc (+tt-inc for DVE chunks),
    # with hT1 at E_HT1, recip0 at E_RC0, mul0 at E_MU0; recip1, mul1 last
    dve_hT = [1, 0]
    dve_u = {}
    dve_tt = {}
    dve_recip = [0, 0]
    dve_mul = [0, 0]
    cnt_d = 1
    for e in range(NE):
        if e == E_HT1:
            cnt_d += 1; dve_hT[1] = cnt_d
        if e == E_RC0:
            cnt_d += 1; dve_recip[0] = cnt_d
        if e == E_MU0:
            cnt_d += 1; dve_mul[0] = cnt_d
        cnt_d += 1
        dve_u[e] = cnt_d
        if not is_gps(e):
            cnt_d += 1
            dve_tt[e] = cnt_d
    cnt_d += 1; dve_recip[1] = cnt_d
    cnt_d += 1; dve_mul[1] = cnt_d

    # sem_gp (inc 1): tt-inc for GPS chunks
    gp_tt = {}
    cnt_g = 0
    for e in range(NE):
        if is_gps(e):
            cnt_g += 1
            gp_tt[e] = cnt_g

    def q_done(e):
        return ("gp", gp_tt[e]) if is_gps(e) else ("dve", dve_tt[e])

    from contextlib import ExitStack
    with ExitStack() as es:
        xt_sb = es.enter_context(nc.sbuf_tensor("xt_sb", [128, KC * N], BF16))
        wp_sb = es.enter_context(nc.sbuf_tensor("wp_sb", [128, H2 * KC * OD], BF16))
        a_sb = es.enter_context(nc.sbuf_tensor("a_sb", [OD, H2], BF16))
        asb_sb = es.enter_context(nc.sbuf_tensor("asb_sb", [OD, H2 * 128], BF16))
        ones_sb = es.enter_context(nc.sbuf_tensor("ones_sb", [1, OD], F32))
        hT_sb = es.enter_context(nc.sbuf_tensor("hT_sb", [OD, H2 * N], BF16))
        hs_sb = es.enter_context(nc.sbuf_tensor("hs_sb", [128, H2 * JC * HSS], BF16))
        g_sb = es.enter_context(nc.sbuf_tensor("g_sb", [128, H2 * JC], F32))
        ed_sb = es.enter_context(nc.sbuf_tensor("ed_sb", [128, H2 * JC], F32))
        gsb_sb = es.enter_context(nc.sbuf_tensor("gsb_sb", [128, H2 * IH], BF16))
        adj_sb = es.enter_context(nc.sbuf_tensor("adj_sb", [128, NBA * IH], BF16))
        u_sb = es.enter_context(nc.sbuf_tensor("u_sb", [128, NBU * IH], BF16))
        q_sb = es.enter_context(nc.sbuf_tensor("q_sb", [128, NBQ * IH], BF16))
        rrow_sb = es.enter_context(nc.sbuf_tensor("rrow_sb", [1, H2 * IH], F32))
        rbc_sb = es.enter_context(nc.sbuf_tensor("rbc_sb", [OD, H2 * IH], F32))
        onorm_sb = es.enter_context(nc.sbuf_tensor("onorm_sb", [OD, H2 * IH], F32))
        pp = [es.enter_context(nc.psum_tensor(f"pp{b}", [128, 512], F32))
              for b in range(6)]
        pf = [es.enter_context(nc.psum_tensor(f"pf{h}", [128, 512], F32))
              for h in range(H2)]
        sem_sp = es.enter_context(nc.semaphore("sem_sp"))
        sem_tp = es.enter_context(nc.semaphore("sem_tp"))
        sem_gp = es.enter_context(nc.semaphore("sem_gp"))
        sem_pe = es.enter_context(nc.semaphore("sem_pe"))
        sem_act = es.enter_context(nc.semaphore("sem_act"))
        sem_dve = es.enter_context(nc.semaphore("sem_dve"))
        block = es.enter_context(nc.Block())

        SEM = {"sp": sem_sp, "tp": sem_tp, "gp": sem_gp,
               "pe": sem_pe, "act": sem_act, "dve": sem_dve}

        def qd_wait(eng, e):
            s, c = q_done(e)
            eng.wait_ge(SEM[s], c)

        # ---------------- SP engine ----------------
        @block.sync
        def _(sync):
            sync.dma_start(
                wp_sb[:, 0:H2 * KC * OD].rearrange(
                    "p (h kc o) -> p h kc o", h=H2, kc=KC),
                wp.rearrange("h (kc p) o -> p h kc o", p=128),
            ).then_inc(sem_sp, 16)
            sync.dma_start(a_sb[:, :], av[:, :]).then_inc(sem_sp, 16)
            sync.dma_start(asb_sb[:, :], asb[:, :]).then_inc(sem_sp, 16)
            for half in range(2):
                sync.dma_start(
                    xt_sb.rearrange("p (kc i) -> p kc i", kc=KC)[
                        :, :, half * IH:(half + 1) * IH],
                    xt.rearrange("(kc p) i -> p kc i", p=128)[
                        :, :, half * IH:(half + 1) * IH],
                ).then_inc(sem_sp, 16)
            adj3 = adj_sb.rearrange("p (c i) -> p c i", c=NBA)
            adjt3 = adjt.rearrange("(c p) i -> p c i", p=128)
            for ab in range(JC // ADJB):
                sync.dma_start(
                    adj3[:, ab * ADJB:(ab + 1) * ADJB, :],
                    adjt3[:, ab * ADJB:(ab + 1) * ADJB, :],
                ).then_inc(sem_sp, 16)
            for h in range(H2):
                sync.wait_ge(sem_dve, dve_mul[h])
                sync.dma_start(
                    out[h], onorm_sb[:, h * IH:(h + 1) * IH]
                ).then_inc(sem_sp, 16)
            if debug:
                for name, sb in [("hT", hT_sb), ("g", g_sb), ("ed", ed_sb),
                                 ("gsb", gsb_sb), ("hs", hs_sb),
                                 ("q", q_sb), ("rec", rbc_sb)]:
                    sync.dma_start(dbg[name][:, :], sb[:, :]).then_inc(sem_sp, 16)

        # ---------------- PE engine ----------------
        @block.tensor
        def _(tensor):
            def proj(h):
                if h == 1:
                    # pp[3..5] hold h0's f_src-bcast until gsb_exps0 reads them
                    tensor.wait_ge(sem_act, act_gsb[0])
                for half in range(2):
                    for kc in range(KC):
                        if h == 0:
                            tensor.wait_ge(sem_sp, sp_xt_done[(kc, half)])
                        for b in range(3 * half, 3 * half + 3):
                            i = tensor.matmul(
                                pp[b][0:OD, :],
                                wp_sb[:, (h * KC + kc) * OD:(h * KC + kc + 1) * OD],
                                xt_sb[:, kc * N + b * 512:kc * N + (b + 1) * 512],
                                start=(kc == 0), stop=(kc == KC - 1),
                            )
                i.then_inc(sem_pe, 1)

            def fcols(h):
                # f_dst columns into pf[h][:, 0:24]
                tensor.wait_ge(sem_act, act_hT[h])
                tensor.wait_ge(sem_dve, dve_hT[h])
                if h == 1:
                    # pf[1] was read by ACT's gsb0 exps
                    tensor.wait_ge(sem_act, act_gsb[0])
                for c in range(JC):
                    i = tensor.matmul(
                        pf[h][:, c:c + 1],
                        hT_sb[:, h * N + c * 128:h * N + (c + 1) * 128],
                        a_sb[:, h:h + 1],
                        start=True, stop=True,
                    )
                i.then_inc(sem_pe, 1)

            def fsbc(h):
                # f_src broadcast rows into pp[3+b] (DVE-half proj banks,
                # free once hT[h]'s DVE copies are done)
                tensor.wait_ge(sem_dve, dve_hT[h])
                for b in range(IBH):
                    i = tensor.matmul(
                        pp[3 + b][:, :],
                        asb_sb[:, h * 128:(h + 1) * 128],
                        hT_sb[:, h * N + b * 512:h * N + (b + 1) * 512],
                        start=True, stop=True,
                    )
                    i.then_inc(sem_pe, 1) if b == IBH - 1 else None

            def bcast(h, b):
                bank = pf[(h + b) % 2]
                if b == 0:
                    tensor.wait_ge(sem_dve, dve_recip[h])
                if b == 2:
                    tensor.wait_ge(sem_act, act_rbc[(h, 0)])
                if h == 1:
                    tensor.wait_ge(sem_act, act_rbc[(0, 2 - b if b < 2 else 0)])
                i = tensor.matmul(
                    bank[0:OD, :], ones_sb[:, :],
                    rrow_sb[0:1, h * IH + b * 512:h * IH + (b + 1) * 512],
                    start=True, stop=True,
                )
                i.then_inc(sem_pe, 1)

            proj(0)
            fcols(0)
            fsbc(0)
            proj(1)
            fcols(1)
            fsbc(1)
            # WAR: agg writes pp banks that hT1 copies still read
            tensor.wait_ge(sem_act, act_hT[1])
            tensor.wait_ge(sem_dve, dve_hT[1])
            for e, (h, jc) in enumerate(CH):
                if (h, jc) == (1, 0):
                    # pp[3..5] held h1's f_src-bcast until gsb_exps1 read them
                    tensor.wait_ge(sem_act, act_gsb[1])
                qd_wait(tensor, e)
                tensor.wait_ge(sem_act, act_hs[e])
                qs = e % NBQ
                for b in range(IBH):
                    i = tensor.matmul(
                        pp[h * 3 + b][0:OD1, :],
                        hs_sb[:, (h * JC + jc) * HSS:(h * JC + jc) * HSS + OD1],
                        q_sb[:, qs * IH + b * 512:qs * IH + (b + 1) * 512],
                        start=(jc == 0), stop=(jc == JC - 1),
                    )
                i.then_inc(sem_pe, 1)
                for b in range(IBH):
                    if e == E_BC0 + b:
                        bcast(0, b)
            for b in range(IBH):
                bcast(1, b)

        # ---------------- ACT engine ----------------
        @block.scalar
        def _(scalar):
            Exp = mybir.ActivationFunctionType.Exp
            Copy = mybir.ActivationFunctionType.Copy

            def hT_copy(h):
                scalar.wait_ge(sem_pe, pe_proj[h])
                for b in range(3):
                    i = scalar.activation(
                        hT_sb[:, h * N + b * 512:h * N + (b + 1) * 512],
                        pp[b][0:OD, :], Copy,
                    )
                i.then_inc(sem_act, 1)

            def transpose_head(h):
                # [64, 3072] -> 24 x [128, 64] chunks at stride HSS
                dst = hs_sb[:, h * JC * HSS:(h * JC + JC) * HSS].rearrange(
                    "p (c o) -> p c o", o=HSS)[:, :, 0:OD]
                scalar.dma_start_transpose(
                    dst, hT_sb[:, h * N:(h + 1) * N],
                ).then_inc(sem_tp, 16)

            def exps(h):
                # g/ed from pf[h] f_dst cols
                scalar.wait_ge(sem_pe, pe_fcols[h])
                scalar.activation(
                    g_sb[:, h * JC:(h + 1) * JC], pf[h][:, 0:JC],
                    Exp, scale=-0.8,
                )
                i = scalar.activation(
                    ed_sb[:, h * JC:(h + 1) * JC], pf[h][:, 0:JC],
                    Exp, scale=1.0,
                )
                i.then_inc(sem_act, 1)

            def gsb_exps(h):
                # Gs broadcast from pp[3+b] f_src-bcast rows
                scalar.wait_ge(sem_pe, pe_fsbc[h])
                for b in range(IBH):
                    i = scalar.activation(
                        gsb_sb[:, h * IH + b * 512:h * IH + (b + 1) * 512],
                        pp[3 + b][:, :], Exp, scale=-0.8,
                    )
                i.then_inc(sem_act, 1)

            def dcopy(h):
                scalar.wait_ge(sem_pe, pe_agg[e_of[(h, JC - 1)]])
                for b in range(IBH):
                    i = scalar.activation(
                        onorm_sb[0:1, h * IH + b * 512:h * IH + (b + 1) * 512],
                        pp[h * 3 + b][OD:OD1, :], Copy,
                    )
                i.then_inc(sem_act, 1)

            def rbc_copy(h, b):
                scalar.wait_ge(sem_pe, pe_bcast[(h, b)])
                scalar.activation(
                    rbc_sb[:, h * IH + b * 512:h * IH + (b + 1) * 512],
                    pf[(h + b) % 2][0:OD, :], Copy,
                ).then_inc(sem_act, 1)

            hT_copy(0)
            scalar.wait_ge(sem_act, act_hT[0])   # own copies retired
            scalar.wait_ge(sem_dve, dve_hT[0])
            transpose_head(0)
            exps(0)
            gsb_exps(0)
            for e, (h, jc) in enumerate(CH):
                if e == H1CP:
                    hT_copy(1)
                    scalar.wait_ge(sem_act, act_hT[1])   # own copies retired
                    scalar.wait_ge(sem_dve, dve_hT[1])
                    transpose_head(1)
                if e == H1P:
                    exps(1)
                    gsb_exps(1)
                if e == E_DC0:
                    dcopy(0)
                for b in range(IBH):
                    if e == E_RB0 + b:
                        rbc_copy(0, b)
                # hs scaling: hs[:, :64] *= Ed, hs[:, 64] = Ed
                base = (h * JC + jc) * HSS
                ecol = ed_sb[:, h * JC + jc:h * JC + jc + 1]
                scalar.wait_ge(sem_tp, tp_trans[h])
                scalar.wait_ge(sem_act, act_exps[h])
                scalar.activation(
                    hs_sb[:, base:base + OD], hs_sb[:, base:base + OD],
                    Copy, scale=ecol,
                )
                scalar.activation(
                    hs_sb[:, base + OD:base + OD1], ecol, Copy,
                ).then_inc(sem_act, 1)
            dcopy(1)
            for b in range(IBH):
                rbc_copy(1, b)

        # ---------------- GP engine: its share of Q muls ----------------
        @block.gpsimd
        def _(gpsimd):
            AL = mybir.AluOpType
            for e, (h, jc) in enumerate(CH):
                if not is_gps(e):
                    continue
                us = e % NBU
                qs = e % NBQ
                gpsimd.wait_ge(sem_dve, dve_u[e])
                gpsimd.wait_ge(sem_sp, sp_adj_done[jc])
                if e >= NBQ:
                    gpsimd.wait_ge(sem_pe, pe_agg[e - NBQ])
                gpsimd.tensor_tensor(
                    q_sb[:, qs * IH:(qs + 1) * IH],
                    u_sb[:, us * IH:(us + 1) * IH],
                    adj_sb[:, jc * IH:(jc + 1) * IH],
                    AL.mult,
                ).then_inc(sem_gp, 1)

        # ---------------- DVE engine ----------------
        @block.vector
        def _(vector):
            AL = mybir.AluOpType
            vector.memset(ones_sb[:, :], 1.0)

            def hT_copy(h):
                vector.wait_ge(sem_pe, pe_proj[h])
                for b in range(3, 6):
                    i = vector.tensor_copy(
                        hT_sb[:, h * N + b * 512:h * N + (b + 1) * 512],
                        pp[b][0:OD, :],
                    )
                i.then_inc(sem_dve, 1)

            def recip(h):
                vector.wait_ge(sem_act, act_dcopy[h])
                vector.reciprocal(
                    rrow_sb[0:1, h * IH:(h + 1) * IH],
                    onorm_sb[0:1, h * IH:(h + 1) * IH],
                ).then_inc(sem_dve, 1)

            def final_mul(h):
                for b in range(IBH):
                    vector.wait_ge(sem_act, act_rbc[(h, b)])
                    i = vector.tensor_mul(
                        onorm_sb[:, h * IH + b * 512:h * IH + (b + 1) * 512],
                        pp[h * 3 + b][0:OD, :],
                        rbc_sb[:, h * IH + b * 512:h * IH + (b + 1) * 512],
                    )
                i.then_inc(sem_dve, 1)

            hT_copy(0)
            for e, (h, jc) in enumerate(CH):
                if e == E_HT1:
                    hT_copy(1)
                if e == E_RC0:
                    recip(0)
                if e == E_MU0:
                    final_mul(0)
                us = e % NBU
                qs = e % NBQ
                # u = max(Gs_bcast * G[j], 1)
                vector.wait_ge(sem_act, act_gsb[h])
                if e >= NBU:
                    qd_wait(vector, e - NBU)
                vector.tensor_scalar(
                    u_sb[:, us * IH:(us + 1) * IH],
                    gsb_sb[:, h * IH:(h + 1) * IH],
                    g_sb[:, h * JC + jc:h * JC + jc + 1],
                    1.0, AL.mult, AL.max,
                ).then_inc(sem_dve, 1)
                if not is_gps(e):
                    vector.wait_ge(sem_sp, sp_adj_done[jc])
                    if e >= NBQ:
                        vector.wait_ge(sem_pe, pe_agg[e - NBQ])
                    vector.tensor_tensor(
                        q_sb[:, qs * IH:(qs + 1) * IH],
                        u_sb[:, us * IH:(us + 1) * IH],
                        adj_sb[:, jc * IH:(jc + 1) * IH],
                        AL.mult,
                    ).then_inc(sem_dve, 1)
            recip(1)
            final_mul(1)

    return nc


_nc_cache = None


def _get_nc():
    global _nc_cache
    if _nc_cache is None:
        _nc_cache = build_program()
    return _nc_cache


def kernel(x, adj, W, a_src, a_dst):
    x = np.asarray(x, np.float32)
    adj = np.asarray(adj, np.float32)
    W = np.asarray(W, np.float32)
    a_src = np.asarray(a_src, np.float32)
    a_dst = np.asarray(a_dst, np.float32)

    bf = ml_dtypes.bfloat16
    swap = np.r_[IH:N, 0:IH]

    xt0 = np.ascontiguousarray(x.T).astype(bf)                    # [KD, N]
    xt1 = np.ascontiguousarray(x.T[:, swap]).astype(bf)
    adjt0 = np.ascontiguousarray(adj[0:IH, :].T).astype(bf)       # [N, IH]
    adjt1 = np.ascontiguousarray(adj[IH:N, :][:, swap].T).astype(bf)

    in_maps = []
    for c in range(N_CORES):
        g, ihf = c // 2, c % 2
        wpair = np.ascontiguousarray(W[2 * g:2 * g + 2]).astype(bf)
        avv = np.stack([a_dst[2 * g], a_dst[2 * g + 1]], axis=1).astype(bf)
        asbv = np.concatenate([
            np.repeat(a_src[2 * g][:, None], 128, axis=1),
            np.repeat(a_src[2 * g + 1][:, None], 128, axis=1)], axis=1).astype(bf)
        in_maps.append({
            "xt": xt0 if ihf == 0 else xt1,
            "adjt": adjt0 if ihf == 0 else adjt1,
            "wp": wpair,
            "av": avv,
            "asb": asbv,
        })

    nc = _get_nc()
    res = run_bass_kernel_spmd(nc, in_maps, list(range(N_CORES)))

    out = np.empty((N, HEADS * OD), np.float32)
    for c in range(N_CORES):
        g, ihf = c // 2, c % 2
        o = res.results[c]["out"]                                 # [2, OD, IH]
        rows = slice(ihf * IH, (ihf + 1) * IH)
        for hh in range(H2):
            hcol = (2 * g + hh) * OD
            out[rows, hcol:hcol + OD] = o[hh].T
    return out
